# revision 8
# baseline (speedup 1.0000x reference)
"""Trainium2 Bass kernel for the gated multi-head attention module.

Math (per batch b, fp32 reference):
  qp = q_b @ Wq.T + bq                      (1024,)    heads (16, 64)
  kp = k_b @ Wq.T + bq                      (512, 1024)
  scores[h, l] = qp_h . kp[l, h-block] / 8  (16, 512)
  qs[h, i] = sum_j scores[h, j] qe[i, j]    (16, 512)
  back = sigmoid(lam*cw + (1-lam)*qs)       (16, 512)
  vp = v_b @ Wv.T + bv                      (512, 3072) -> (512, 1024)
  vg[l, o] = back[o//64, l] * vp[l, o]
  out = vg @ Wo.T + bo                      (512, 1024)
Outputs: (out, qs[head 0])

Device strategy: data-parallel, 8 batches per NeuronCore.  All activations are
kept feature-on-partition ("transposed" layout) so every projection contracts
over the SBUF partition dim.  Host pre-transposes k/v/qe once; big matmuls run
as float32r (full PE rate at moving-dim 512).  Partition-dim reductions and
head-broadcasts are done with tiny constant 0/1 selector matmuls on the PE.
"""

import os
import sys

import numpy as np

for _p in ("/opt/trn_rl_repo",):
    if _p not in sys.path and not any("trn_rl_repo" in p for p in sys.path):
        sys.path.append(_p)

from contextlib import ExitStack

import concourse.bass as bass
import concourse.tile as tile
from concourse import bacc
from concourse import mybir
from concourse.bass_utils import run_bass_kernel_spmd
from concourse.masks import make_identity

BS, L, D, H, DK = 64, 512, 1024, 16, 64
NCORES = 8
NB = BS // NCORES          # batches per core
OC = D // 128              # 8 output-dim chunks
FCK = (2 * D) // 128       # 16 contraction chunks for Wq
FCV = (3 * D) // 128       # 24 contraction chunks for Wv
JC = L // 128              # 4 chunks of the L dim
FD = mybir.dt.float32
FR = mybir.dt.float32r
AOP = mybir.AluOpType
AF = mybir.ActivationFunctionType

_CACHE = {}


def _build_bass():
    nc = bacc.Bacc()

    kT = nc.dram_tensor("kT", [NB, 2 * D, L], FD, kind="ExternalInput")
    vT = nc.dram_tensor("vT", [NB, 3 * D, L], FD, kind="ExternalInput")
    qeT = nc.dram_tensor("qeT", [NB, L, L], FD, kind="ExternalInput")
    cw = nc.dram_tensor("cw", [NB, L], FD, kind="ExternalInput")
    qT = nc.dram_tensor("qT", [2 * D, NB], FD, kind="ExternalInput")
    wqT = nc.dram_tensor("wqT", [2 * D, D], FD, kind="ExternalInput")
    wvT = nc.dram_tensor("wvT", [3 * D, D], FD, kind="ExternalInput")
    woT = nc.dram_tensor("woT", [D, D], FD, kind="ExternalInput")
    bqD = nc.dram_tensor("bq", [D], FD, kind="ExternalInput")
    bvD = nc.dram_tensor("bv", [D], FD, kind="ExternalInput")
    boD = nc.dram_tensor("bo", [D], FD, kind="ExternalInput")
    lamD = nc.dram_tensor("lam", [H, 2], FD, kind="ExternalInput")
    e2D = nc.dram_tensor("e2", [D, H], FD, kind="ExternalInput")
    etD = nc.dram_tensor("et", [H, D], FD, kind="ExternalInput")

    outT = nc.dram_tensor("outT", [NB, D, L], FD, kind="ExternalOutput")
    fcw = nc.dram_tensor("fcw", [NB, L], FD, kind="ExternalOutput")

    back_dram = nc.dram_tensor("back_bounce", [NB, H, L], FR)

    with tile.TileContext(nc) as tc, ExitStack() as top:
        consts = top.enter_context(tc.tile_pool(name="consts", bufs=1))

        ident = consts.tile([128, 128], FD)
        make_identity(nc, ident)
        e2_sb = consts.tile([128, OC, H], FR)
        nc.sync.dma_start(e2_sb, e2D.rearrange("(oc p) h -> p oc h", p=128).bitcast(FR))
        et_sb = consts.tile([H, D], FR)
        nc.sync.dma_start(et_sb, etD[:, :].bitcast(FR))
        lam_sb = consts.tile([H, 2], FD)
        nc.sync.dma_start(lam_sb, lamD[:, :])
        bq_sb = consts.tile([128, OC], FD)
        nc.sync.dma_start(bq_sb, bqD.rearrange("(oc p) -> p oc", p=128))
        bv_sb = consts.tile([128, OC], FD)
        nc.sync.dma_start(bv_sb, bvD.rearrange("(oc p) -> p oc", p=128))
        bo_sb = consts.tile([128, OC], FD)
        nc.sync.dma_start(bo_sb, boD.rearrange("(oc p) -> p oc", p=128))
        qT_sb = consts.tile([128, FCK, NB], FR)
        nc.sync.dma_start(qT_sb, qT.rearrange("(fc p) b -> p fc b", p=128).bitcast(FR))
        qpf8 = consts.tile([128, OC, NB], FD)

        # ---------------- Phase A: kp projection, scores, gate ----------------
        with ExitStack() as phA:
            wq_pool = phA.enter_context(tc.tile_pool(name="wq", bufs=1))
            wq_sb = wq_pool.tile([128, FCK, D], FR)
            wq_re = wqT.rearrange("(fc p) o -> p fc o", p=128).bitcast(FR)
            for g in range(8):
                nc.sync.dma_start(
                    wq_sb[:, 2 * g : 2 * g + 2, :], wq_re[:, 2 * g : 2 * g + 2, :]
                )
            kt_pool = phA.enter_context(tc.tile_pool(name="kt", bufs=2))
            qe_pool = phA.enter_context(tc.tile_pool(name="qe", bufs=3))
            tmpA_pool = phA.enter_context(tc.tile_pool(name="tmpA", bufs=3))
            smallA = phA.enter_context(tc.tile_pool(name="smallA", bufs=4))
            ps_kp = phA.enter_context(tc.tile_pool(name="ps_kp", bufs=3, space="PSUM"))
            ps_sc = phA.enter_context(tc.tile_pool(name="ps_sc", bufs=2, space="PSUM"))
            ps_mi = phA.enter_context(tc.tile_pool(name="ps_mi", bufs=2, space="PSUM"))

            # qp for all 8 batches: (8, 1024) then transpose to (128, oc, b)
            qp_ps0 = ps_mi.tile([NB, 512], FD, tag="mi")
            qp_ps1 = ps_mi.tile([NB, 512], FD, tag="mi")
            for fc in range(FCK):
                st, sp = fc == 0, fc == FCK - 1
                nc.tensor.matmul(
                    qp_ps0, (qT_sb[:, fc, :]), (wq_sb[:, fc, 0:512]),
                    start=st, stop=sp,
                )
                nc.tensor.matmul(
                    qp_ps1, (qT_sb[:, fc, :]), (wq_sb[:, fc, 512:1024]),
                    start=st, stop=sp,
                )
            qp_sbt = smallA.tile([NB, D], FD, bufs=1)
            nc.scalar.copy(qp_sbt[:, 0:512], qp_ps0)
            nc.scalar.copy(qp_sbt[:, 512:1024], qp_ps1)
            for oc in range(OC):
                qpT_ps = ps_mi.tile([128, NB], FD, name="qpT_ps", tag="mi")
                nc.tensor.transpose(
                    qpT_ps, qp_sbt[:, oc * 128 : (oc + 1) * 128], ident[0:NB, 0:NB]
                )
                # qpf8 = (qp + bq) / 8
                nc.vector.tensor_scalar(
                    qpf8[:, oc, :], qpT_ps, bq_sb[:, oc : oc + 1], 0.125,
                    AOP.add, AOP.mult,
                )

            for b in range(NB):
                kt = kt_pool.tile([128, FCK, L], FR, name="kt")
                kt_re = kT[b].rearrange("(fc p) l -> p fc l", p=128).bitcast(FR)
                for g in range(4):
                    nc.sync.dma_start(
                        kt[:, 4 * g : 4 * g + 4, :], kt_re[:, 4 * g : 4 * g + 4, :]
                    )
                sc_ps = ps_sc.tile([H, L], FD, name="sc_ps")
                for oc in range(OC):
                    kp_ps = ps_kp.tile([128, L], FD, name="kp_ps")
                    for fc in range(FCK):
                        nc.tensor.matmul(
                            kp_ps,
                            (wq_sb[:, fc, oc * 128 : (oc + 1) * 128]),
                            (kt[:, fc, :]),
                            start=(fc == 0), stop=(fc == FCK - 1),
                        )
                    # tmp = (kp + bq) * qp/8   (per-partition scalars)
                    tmp = tmpA_pool.tile([128, L], FR, name="tmp")
                    nc.vector.tensor_scalar(
                        tmp, kp_ps, bq_sb[:, oc : oc + 1], qpf8[:, oc, b : b + 1],
                        AOP.add, AOP.mult,
                    )
                    # scores += E2[oc].T @ tmp  (reduce over partitions)
                    nc.tensor.matmul(
                        sc_ps, (e2_sb[:, oc, :]), (tmp),
                        start=(oc == 0), stop=(oc == OC - 1),
                    )
                sc_sb = smallA.tile([H, L], FD, name="sc_sb", bufs=2)
                nc.scalar.copy(sc_sb, sc_ps)
                s_sb = smallA.tile([128, JC, H], FR, name="s_sb", bufs=2)
                for jc in range(JC):
                    s_ps = ps_mi.tile([128, H], FD, name="s_ps", tag="mi")
                    nc.tensor.transpose(
                        s_ps, sc_sb[:, jc * 128 : (jc + 1) * 128], ident[0:H, 0:H]
                    )
                    nc.vector.tensor_copy(s_sb[:, jc, :], s_ps)
                qet = qe_pool.tile([128, JC, L], FR, name="qet")
                qe_re = qeT[b].rearrange("(jc p) l -> p jc l", p=128).bitcast(FR)
                for g in range(2):
                    nc.sync.dma_start(
                        qet[:, 2 * g : 2 * g + 2, :], qe_re[:, 2 * g : 2 * g + 2, :]
                    )
                qs_ps = ps_sc.tile([H, L], FD, name="qs_ps", bufs=1)
                for jc in range(JC):
                    nc.tensor.matmul(
                        qs_ps, (s_sb[:, jc, :]), (qet[:, jc, :]),
                        start=(jc == 0), stop=(jc == JC - 1),
                    )
                # head-0 row of qe_score is the second output
                fcw_t = smallA.tile([1, L], FD, name="fcw_t", bufs=2)
                nc.scalar.copy(fcw_t, qs_ps[0:1, :])
                nc.sync.dma_start(fcw[b], fcw_t)

                cwb = smallA.tile([H, L], FD, name="cwb", bufs=2)
                cw_src = cw[b]
                cw_bc = bass.AP(
                    tensor=cw_src.tensor, offset=cw_src.offset,
                    ap=[[0, H]] + list(cw_src.ap),
                )
                nc.sync.dma_start(cwb, cw_bc)
                t1 = smallA.tile([H, L], FD, name="t1", bufs=2)
                nc.vector.tensor_scalar(t1, cwb, lam_sb[:, 0:1], None, AOP.mult)
                t3 = smallA.tile([H, L], FD, name="t3", bufs=2)
                nc.vector.scalar_tensor_tensor(
                    t3, qs_ps, lam_sb[:, 1:2], t1, AOP.mult, AOP.add
                )
                back_t = smallA.tile([H, L], FR, name="back_t", bufs=2)
                nc.scalar.activation(back_t, t3, AF.Sigmoid)
                nc.sync.dma_start(back_dram[b], back_t)

        # ---------------- Phase B: vp projection, gate, output ----------------
        with ExitStack() as phB:
            wv_pool = phB.enter_context(tc.tile_pool(name="wv", bufs=1))
            wv_sb = wv_pool.tile([128, FCV, D], FR)
            wv_re = wvT.rearrange("(fc p) o -> p fc o", p=128).bitcast(FR)
            for g in range(12):
                nc.sync.dma_start(
                    wv_sb[:, 2 * g : 2 * g + 2, :], wv_re[:, 2 * g : 2 * g + 2, :]
                )
            wo_pool = phB.enter_context(tc.tile_pool(name="wo", bufs=1))
            wo_sb = wo_pool.tile([128, OC, D], FR)
            wo_re = woT.rearrange("(oc p) o -> p oc o", p=128).bitcast(FR)
            for g in range(4):
                nc.sync.dma_start(
                    wo_sb[:, 2 * g : 2 * g + 2, :], wo_re[:, 2 * g : 2 * g + 2, :]
                )
            vt_pool = phB.enter_context(tc.tile_pool(name="vt", bufs=3))
            vg_pool = phB.enter_context(tc.tile_pool(name="vg", bufs=1))
            tmpB_pool = phB.enter_context(tc.tile_pool(name="tmpB", bufs=3))
            outsb_pool = phB.enter_context(tc.tile_pool(name="outsb", bufs=3))
            smallB = phB.enter_context(tc.tile_pool(name="smallB", bufs=2))
            ps_vp = phB.enter_context(tc.tile_pool(name="ps_vp", bufs=4, space="PSUM"))
            ps_bx = phB.enter_context(tc.tile_pool(name="ps_bx", bufs=2, space="PSUM"))
            ps_ou = phB.enter_context(tc.tile_pool(name="ps_ou", bufs=2, space="PSUM"))

            for b in range(NB):
                back_b = smallB.tile([H, L], FR, name="back_b")
                nc.sync.dma_start(back_b, back_dram[b])
                vg_sb = vg_pool.tile([128, OC, L], FR, name="vg_sb")
                vt_re = vT[b].rearrange("(fc p) l -> p fc l", p=128).bitcast(FR)
                for half in range(2):
                    vp_ps = [
                        ps_vp.tile([128, L], FD, name="vp_ps") for _ in range(4)
                    ]
                    for fcg in range(FCV // 2):
                        vt = vt_pool.tile([128, 2, L], FR, name="vt")
                        nc.sync.dma_start(
                            vt, vt_re[:, 2 * fcg : 2 * fcg + 2, :]
                        )
                        for fcs in range(2):
                            fc = 2 * fcg + fcs
                            for i in range(4):
                                oc = half * 4 + i
                                nc.tensor.matmul(
                                    vp_ps[i],
                                    (wv_sb[:, fc, oc * 128 : (oc + 1) * 128]),
                                    (vt[:, fcs, :]),
                                    start=(fc == 0), stop=(fc == FCV - 1),
                                )
                    for i in range(4):
                        oc = half * 4 + i
                        # broadcast gate over the 64 rows of each head block
                        bx_ps = ps_bx.tile([128, L], FD, name="bx_ps")
                        nc.tensor.matmul(
                            bx_ps,
                            (et_sb[:, oc * 128 : (oc + 1) * 128]),
                            (back_b),
                        )
                        vp_t = tmpB_pool.tile([128, L], FD, name="vp_t")
                        nc.scalar.add(vp_t, vp_ps[i], bv_sb[:, oc : oc + 1])
                        nc.vector.tensor_mul(vg_sb[:, oc, :], vp_t, bx_ps)
                # output projection
                for pc in range(OC):
                    ou_ps = ps_ou.tile([128, L], FD, name="ou_ps")
                    for oc in range(OC):
                        nc.tensor.matmul(
                            ou_ps,
                            (wo_sb[:, oc, pc * 128 : (pc + 1) * 128]),
                            (vg_sb[:, oc, :]),
                            start=(oc == 0), stop=(oc == OC - 1),
                        )
                    ou_sb = outsb_pool.tile([128, L], FD, name="ou_sb")
                    nc.scalar.add(ou_sb, ou_ps, bo_sb[:, pc : pc + 1])
                    nc.sync.dma_start(outT[b, pc * 128 : (pc + 1) * 128, :], ou_sb)

    nc.finalize()
    return nc


def _get_nc():
    if "nc" not in _CACHE:
        _CACHE["nc"] = _build_bass()
    return _CACHE["nc"]


def _prepare_in_maps(q, k, v_backward, correlation_weight, qe, Wq, bq, Wv, bv,
                     Wo, bo, lambdas):
    q = np.asarray(q, dtype=np.float32)
    k = np.asarray(k, dtype=np.float32)
    v_backward = np.asarray(v_backward, dtype=np.float32)
    correlation_weight = np.asarray(correlation_weight, dtype=np.float32)
    qe = np.asarray(qe, dtype=np.float32)
    Wq = np.asarray(Wq, dtype=np.float32)
    bq = np.asarray(bq, dtype=np.float32)
    Wv = np.asarray(Wv, dtype=np.float32)
    bv = np.asarray(bv, dtype=np.float32)
    Wo = np.asarray(Wo, dtype=np.float32)
    bo = np.asarray(bo, dtype=np.float32)
    lam = np.asarray(lambdas, dtype=np.float32).reshape(H)

    wqT = np.ascontiguousarray(Wq.T)           # (2048, 1024)
    wvT = np.ascontiguousarray(Wv.T)           # (3072, 1024)
    woT = np.ascontiguousarray(Wo.T)           # (1024, 1024)
    lam2 = np.stack([lam, 1.0 - lam], axis=1)  # (16, 2)
    oidx = np.arange(D) // DK
    e2 = (oidx[:, None] == np.arange(H)[None, :]).astype(np.float32)  # (1024,16)
    et = np.ascontiguousarray(e2.T)                                   # (16,1024)

    kT_all = np.ascontiguousarray(np.swapaxes(k, 1, 2))           # (64,2048,512)
    vT_all = np.ascontiguousarray(np.swapaxes(v_backward, 1, 2))  # (64,3072,512)
    qeT_all = np.ascontiguousarray(np.swapaxes(qe, 1, 2))         # (64,512,512)
    q2 = q.reshape(BS, 2 * D)

    in_maps = []
    for c in range(NCORES):
        sl = slice(c * NB, (c + 1) * NB)
        in_maps.append({
            "kT": kT_all[sl],
            "vT": vT_all[sl],
            "qeT": qeT_all[sl],
            "cw": np.ascontiguousarray(correlation_weight[sl]),
            "qT": np.ascontiguousarray(q2[sl].T),
            "wqT": wqT, "wvT": wvT, "woT": woT,
            "bq": bq, "bv": bv, "bo": bo,
            "lam": lam2, "e2": e2, "et": et,
        })
    return in_maps


def run(trace=False, **inputs):
    nc = _get_nc()
    in_maps = _prepare_in_maps(**inputs)
    res = run_bass_kernel_spmd(
        nc, in_maps, core_ids=list(range(NCORES)), trace=trace,
    )
    out = np.empty((BS, L, D), dtype=np.float32)
    fcw = np.empty((BS, L), dtype=np.float32)
    for c in range(NCORES):
        r = res.results[c]
        out[c * NB : (c + 1) * NB] = np.swapaxes(r["outT"], 1, 2)
        fcw[c * NB : (c + 1) * NB] = r["fcw"]
    return (out, fcw), res


def kernel(**inputs):
    (out, fcw), _ = run(trace=False, **inputs)
    return out, fcw
